# revision 7
# baseline (speedup 1.0000x reference)
"""Trainium2 Bass kernel for nn_DEA1D (convolutional FISTA / learned sparse coding).

Math (per batch element, reference semantics):
    D_enc = 4093, K = 16, stride = 4, 64 filters, T = 20 FISTA iterations.
    step:  x_pre = yk + conv(x - conv_t(yk), H)/L
           x_new = softshrink(x_pre, thr)
           yk    = x_new + beta_k (x_new - x_old)
    out:   z = conv_t(x_new), x_new, lam

Device formulation: precompute cx = conv(x)/L once, then iterate entirely in
code space with the Gram operator expanded into 7 banded 64x64 matrices A_d
(d = j-shift in -3..3):  x_pre = cx + (I - A_0/L) yk - sum_{d!=0} (A_d/L) yk_d
All per-iteration matmuls run in float32r (TF32-like, 1 col/cycle on the PE);
two batch elements are packed per 128 partitions (64 channels each).

Sharding: data-parallel over batch: 32 batches / 8 cores = 4 per core,
processed as 2 groups of 2. The tiny dictionary H is replicated.
"""

import sys
import numpy as np

for _p in ("/opt/trn_rl_repo", "/opt/pypackages"):
    if _p not in sys.path:
        sys.path.append(_p)

# ---------------------------------------------------------------- constants
B_TOT = 32
N_CORES = 8
NB = 4                      # batches per core
NG = 2                      # groups of 2 batches per core
D_IN = 16384
KK = 16                     # kernel taps
ST = 4                      # stride
F = 64                      # num filters
D_ENC = (D_IN - KK) // ST + 1            # 4093
T_ITERS = 20
L = 10.0
SIGMA = 0.1
PADL = 4                    # zero pad columns on each side of code tiles
WCOL = PADL + D_ENC + PADL  # 4101
M_OUT = D_IN // ST          # 4096 phase-major output length

# chunks of the D_ENC axis, each fits one PSUM bank (<=512 fp32).
# fp32r matmuls require an even moving-dim size, so the last chunk runs a
# 510-wide matmul (one dummy column) but only 509 columns are drained.
CHUNKS = [(i * 512, 512, 512) for i in range(7)] + [(3584, 510, 509)]
ZCH = [(i * 512, 512) for i in range(8)]   # chunks of the 4096 m axis

_DELTAS = [0, 1, -1, 2, -2, 3, -3]


def _np_f32(x):
    return np.float32(x)


def _fista_consts():
    """thr and per-iteration momentum coefficients, in float32 like the ref."""
    lam = _np_f32(SIGMA) * np.sqrt(_np_f32(2.0) * np.log(_np_f32(F * D_ENC)))
    thr = lam / _np_f32(L)
    betas = {}
    t = _np_f32(1.0)
    for k in range(1, T_ITERS + 1):
        t_new = (_np_f32(1.0) + np.sqrt(_np_f32(1.0) + _np_f32(4.0) * t * t)) / _np_f32(2.0)
        betas[k] = (t - _np_f32(1.0)) / t_new
        t = t_new
    return lam, thr, betas


# ------------------------------------------------------- walrus legalization
def _split_multi_waits(nc, mybir):
    """This walrus build allows only one sync wait per instruction: hoist
    extra waits onto same-engine InstNoOps inserted immediately before."""
    cnt = 0
    for fn in nc.m.functions:
        for bb in fn.blocks:
            out = []
            changed = False
            for ins in bb.instructions:
                si = ins.sync_info
                waits = list(si.on_wait) if (si is not None and si.on_wait) else []
                if len(waits) > 1:
                    changed = True
                    for w in waits[:-1]:
                        cnt += 1
                        nop = mybir.InstNoOp(name=f"waitsplit_{cnt}", ins=[], outs=[])
                        nop.engine = ins.engine
                        nop.sync_info = mybir.SyncInfo(on_wait=[w], on_update=[])
                        out.append(nop)
                    ins.sync_info = mybir.SyncInfo(
                        on_wait=[waits[-1]],
                        on_update=list(si.on_update) if si.on_update else [])
                out.append(ins)
            if changed:
                bb.instructions = out
    return cnt


# ------------------------------------------------------------ device program
def _build_program():
    import concourse.bass as bass
    import concourse.tile as tile
    import concourse.mybir as mybir
    from concourse.masks import make_identity

    f32 = mybir.dt.float32
    f32r = mybir.dt.float32r
    Alu = mybir.AluOpType
    Act = mybir.ActivationFunctionType

    _, thr, betas = _fista_consts()
    thr = float(thr)
    inv_l = -1.0 / L

    nc = bass.Bass(trn_type="TRN2")
    xf_d = nc.dram_tensor("xF", [NG, 2 * KK, D_ENC], f32, kind="ExternalInput")
    h_d = nc.dram_tensor("Hm", [F, KK], f32, kind="ExternalInput")
    ht_d = nc.dram_tensor("HTm", [KK, F], f32, kind="ExternalInput")
    xo_d = nc.dram_tensor("xo", [NG, 128, D_ENC], f32, kind="ExternalOutput")
    zo_d = nc.dram_tensor("zo", [NG, 2 * ST, M_OUT], f32, kind="ExternalOutput")

    with tile.TileContext(nc) as tc:
        with tc.tile_pool(name="wpool", bufs=1) as wp, \
             tc.tile_pool(name="state", bufs=1) as stp, \
             tc.tile_pool(name="tmps", bufs=2) as tp, \
             tc.tile_pool(name="psum", bufs=8, space="PSUM") as pp:

            # ---------------- dictionary-derived weights ----------------
            h_sb = wp.tile([F, KK], f32, tag="h_sb")
            ht_sb = wp.tile([KK, F], f32, tag="ht_sb")
            nc.sync.dma_start(h_sb[:], h_d[:])
            nc.sync.dma_start(ht_sb[:], ht_d[:])

            z128 = wp.tile([128, 128], f32, tag="z128")
            nc.vector.memset(z128[:], 0.0)
            negthr = wp.tile([128, 1], f32, tag="negthr")
            nc.vector.memset(negthr[:], -thr)
            id64 = wp.tile([64, 64], f32, tag="id64")
            make_identity(nc, id64[:])
            id128 = wp.tile([128, 128], f32, tag="id128")
            make_identity(nc, id128[:])

            # staged shifted copies of H^T (matmul operands need base partition 0)
            ht_sh = {0: ht_sb}
            for s in (4, 8, 12):
                t_ = wp.tile([KK, F], f32, tag=f"ht_sh{s}", name=f"ht_sh{s}")
                nc.sync.dma_start(t_[0:KK - s, :], ht_sb[s:KK, :])
                ht_sh[s] = t_

            # W_t: blockdiag(B_d, B_d), B_d = (d==0 ? I : 0) - A_d / L, f32r
            w_gram = []
            for t_idx, d in enumerate(_DELTAS):
                s = 4 * abs(d)
                a_ps = pp.tile([128, 512], f32, tag="ps", name=f"aps{t_idx}")
                if d >= 0:
                    lhsT, rhs = ht_sb[0:KK - s, :], ht_sh[s][0:KK - s, :]
                else:
                    lhsT, rhs = ht_sh[s][0:KK - s, :], ht_sb[0:KK - s, :]
                nc.tensor.matmul(a_ps[:64, :64], lhsT, rhs, start=True, stop=True)
                tmp64 = tp.tile([64, 64], f32, tag="tmp64")
                nc.scalar.activation(tmp64[:], a_ps[:64, :64], Act.Copy, scale=inv_l)
                if d == 0:
                    nc.vector.tensor_tensor(tmp64[:], tmp64[:], id64[:], Alu.add)
                w = wp.tile([128, 128], f32r, tag=f"wg{t_idx}", name=f"wg{t_idx}")
                nc.vector.tensor_copy(w[:], z128[:])
                nc.vector.tensor_copy(w[0:64, 0:64], tmp64[:])
                nc.vector.tensor_copy(w[64:128, 64:128], tmp64[:])
                w_gram.append(w)

            w_id = wp.tile([128, 128], f32r, tag="w_id")
            nc.vector.tensor_copy(w_id[:], id128[:])

            # W_H = blockdiag(H^T/L, H^T/L), fp32 (used once, full precision)
            hl = tp.tile([KK, F], f32, tag="hl")
            nc.scalar.activation(hl[:], ht_sb[:], Act.Copy, scale=1.0 / L)
            w_h = wp.tile([2 * KK, 128], f32, tag="w_h")
            nc.vector.tensor_copy(w_h[:], z128[0:2 * KK, :])
            nc.vector.tensor_copy(w_h[0:KK, 0:F], hl[:])
            nc.sync.dma_start(w_h[KK:2 * KK, F:128], hl[:])

            # WZ_d [128, 8] f32r: blockdiag of H[:, 4d:4d+4] (final conv_t)
            w_z = []
            for d in range(ST):
                wz = wp.tile([128, 2 * ST], f32r, tag=f"wz{d}", name=f"wz{d}")
                nc.vector.tensor_copy(wz[:], z128[:, 0:2 * ST])
                nc.vector.tensor_copy(wz[0:F, 0:ST], h_sb[:, 4 * d:4 * d + 4])
                nc.vector.tensor_copy(wz[F:128, ST:2 * ST], h_sb[:, 4 * d:4 * d + 4])
                w_z.append(wz)

            # ---------------- state tiles (per group) ----------------
            yk = [[stp.tile([128, WCOL], f32r, tag=f"yk{p}{g}", name=f"yk{p}{g}") for g in range(NG)]
                  for p in range(2)]
            xs = [[stp.tile([128, WCOL], f32r, tag=f"xs{p}{g}", name=f"xs{p}{g}") for g in range(NG)]
                  for p in range(2)]
            cxl = [stp.tile([128, D_ENC + 1], f32r, tag=f"cxl{g}", name=f"cxl{g}") for g in range(NG)]

            # zero the pads (f32r tiles can't be memset: copy from fp32 zeros)
            for p in range(2):
                for g in range(NG):
                    for t_ in (yk[p][g], xs[p][g]):
                        nc.vector.tensor_copy(t_[:, 0:PADL], z128[:, 0:PADL])
                        nc.vector.tensor_copy(t_[:, PADL + D_ENC:WCOL], z128[:, 0:PADL])

            # ---------------- cx = conv(x)/L and iteration 1 ----------------
            # x1 = yk1 = softshrink(cx); softshrink(v) = relu(v-thr) - relu(-v-thr)
            for g in range(NG):
                nc.vector.tensor_copy(cxl[g][:, D_ENC:D_ENC + 1], z128[:, 0:1])
                for (c0, _nm, n) in CHUNKS:
                    fx = tp.tile([2 * KK, 512], f32, tag="fx")
                    nc.sync.dma_start(fx[:, :n], xf_d[g, :, c0:c0 + n])
                    ps = pp.tile([128, 512], f32, tag="ps", name=f"cx{g}_{c0}")
                    nc.tensor.matmul(ps[:, :n], w_h[:], fx[:, :n], start=True, stop=True)
                    nc.scalar.activation(cxl[g][:, c0:c0 + n], ps[:, :n], Act.Copy)
                    av = tp.tile([128, 512], f32, tag="av")
                    bv = tp.tile([128, 512], f32, tag="bv")
                    nc.scalar.activation(av[:, :n], ps[:, :n], Act.Relu, bias=negthr[:], scale=1.0)
                    nc.scalar.activation(bv[:, :n], ps[:, :n], Act.Relu, bias=negthr[:], scale=-1.0)
                    nc.vector.tensor_tensor(
                        xs[0][g][:, PADL + c0:PADL + c0 + n], av[:, :n], bv[:, :n], Alu.subtract)
                    nc.vector.tensor_tensor(
                        yk[0][g][:, PADL + c0:PADL + c0 + n], av[:, :n], bv[:, :n], Alu.subtract)

            # ---------------- FISTA iterations 2..T ----------------
            for k in range(2, T_ITERS + 1):
                beta = float(betas[k])
                rd, wr = k % 2, (k + 1) % 2    # iter1 wrote slot 0; k=2 reads 0
                for g in range(NG):
                    ykr, ykw = yk[rd][g], yk[wr][g]
                    xr, xw = xs[rd][g], xs[wr][g]
                    for h in range(2):
                        chs = CHUNKS[4 * h:4 * h + 4]
                        ps_t = {}
                        for (c0, nm, n) in chs:
                            ps_t[c0] = pp.tile([128, 512], f32, tag="ps", name=f"it{k}g{g}c{c0}")
                        # t-outer: weight reused across the 4 chunks
                        for (c0, nm, n) in chs:
                            nc.tensor.matmul(ps_t[c0][:, :nm], w_id[:],
                                             cxl[g][:, c0:c0 + nm],
                                             start=True, stop=False)
                        for t_idx, d in enumerate(_DELTAS):
                            last = t_idx == len(_DELTAS) - 1
                            for (c0, nm, n) in chs:
                                nc.tensor.matmul(
                                    ps_t[c0][:, :nm], w_gram[t_idx],
                                    ykr[:, PADL + d + c0:PADL + d + c0 + nm],
                                    start=False, stop=last)
                        for (c0, nm, n) in chs:
                            ps = ps_t[c0]
                            av = tp.tile([128, 512], f32, tag="av")
                            bv = tp.tile([128, 512], f32, tag="bv")
                            nc.scalar.activation(av[:, :n], ps[:, :n], Act.Relu,
                                                 bias=negthr[:], scale=1.0)
                            nc.scalar.activation(bv[:, :n], ps[:, :n], Act.Relu,
                                                 bias=negthr[:], scale=-1.0)
                            nc.vector.tensor_tensor(
                                xw[:, PADL + c0:PADL + c0 + n],
                                av[:, :n], bv[:, :n], Alu.subtract)
                            if k < T_ITERS:
                                tmv = tp.tile([128, 512], f32, tag="tmv")
                                nc.gpsimd.tensor_scalar_mul(
                                    tmv[:, :n],
                                    xr[:, PADL + c0:PADL + c0 + n].bitcast(f32),
                                    -beta)
                                nc.vector.scalar_tensor_tensor(
                                    ykw[:, PADL + c0:PADL + c0 + n],
                                    xw[:, PADL + c0:PADL + c0 + n].bitcast(f32),
                                    1.0 + beta, tmv[:, :n], Alu.mult, Alu.add)

            xfin = [xs[(T_ITERS + 1) % 2][g] for g in range(NG)]

            # ---------------- z = conv_t(x_final), phase-major ----------------
            for g in range(NG):
                zsb = stp.tile([2 * ST, M_OUT], f32, tag="zsb", name=f"zsb{g}")
                for (c0, n) in ZCH:
                    ps = pp.tile([128, 512], f32, tag="ps", name=f"z{g}_{c0}")
                    for d in range(ST):
                        nc.tensor.matmul(
                            ps[:2 * ST, :n], w_z[d],
                            xfin[g][:, PADL + c0 - d:PADL + c0 - d + n],
                            start=(d == 0), stop=(d == ST - 1))
                    nc.scalar.activation(zsb[:, c0:c0 + n], ps[:2 * ST, :n], Act.Copy)
                nc.sync.dma_start(zo_d[g], zsb[:])
                nc.sync.dma_start(xo_d[g], xfin[g][:, PADL:PADL + D_ENC].bitcast(f32))

    _split_multi_waits(nc, mybir)
    return nc


_CACHE = {}


def _get_nc():
    if "nc" not in _CACHE:
        _CACHE["nc"] = _build_program()
    return _CACHE["nc"]


def _make_in_maps(x, H):
    """x [32,1,16384], H [64,1,16] -> per-core input dicts."""
    x2 = np.ascontiguousarray(x.reshape(B_TOT, D_IN).astype(np.float32, copy=False))
    h2 = np.ascontiguousarray(H.reshape(F, KK).astype(np.float32, copy=False))
    ht = np.ascontiguousarray(h2.T)
    in_maps = []
    for c in range(N_CORES):
        xf = np.empty((NG, 2 * KK, D_ENC), dtype=np.float32)
        for g in range(NG):
            for s in range(2):
                b = NB * c + 2 * g + s
                fr = np.lib.stride_tricks.sliding_window_view(x2[b], KK)[::ST]  # [D_ENC, K]
                xf[g, s * KK:(s + 1) * KK, :] = fr.T
        in_maps.append({"xF": xf, "Hm": h2, "HTm": ht})
    return in_maps


def kernel(x, H):
    from concourse.bass_utils import run_bass_kernel_spmd

    x = np.asarray(x)
    H = np.asarray(H)
    nc = _get_nc()
    in_maps = _make_in_maps(x, H)
    res = run_bass_kernel_spmd(nc, in_maps, core_ids=list(range(N_CORES)))
    results = res.results

    x_new = np.empty((B_TOT, F, D_ENC), dtype=np.float32)
    z = np.empty((B_TOT, 1, D_IN), dtype=np.float32)
    for c in range(N_CORES):
        xo = results[c]["xo"]          # [NG, 128, D_ENC]
        zo = results[c]["zo"]          # [NG, 8, M_OUT]
        for g in range(NG):
            for s in range(2):
                b = NB * c + 2 * g + s
                x_new[b] = xo[g, s * F:(s + 1) * F, :]
                z[b, 0] = zo[g, s * ST:(s + 1) * ST, :].T.reshape(D_IN)

    lam, _, _ = _fista_consts()
    return z, x_new, np.float32(lam)
